# revision 11
# baseline (speedup 1.0000x reference)
"""Trainium2 Bass kernel for nn_Backbone_4243427688698 (gnn_message_passing).

Strategy (data-parallel over graphs, 8 cores):
  - Each of the 8 cores processes 2048 whole 9-node graphs.
  - Edges never cross graphs, so message passing for a group of 14 graphs
    (126 nodes, padded to 128) is a dense block-diagonal 128x128 adjacency
    matmul on the TensorEngine.
  - The full 4-layer GCN for a block is fused in SBUF: only the initial node
    embeddings h0 and the normalized adjacency blocks stream in from HBM; the
    only output is the [B, 1] sigmoid readout.
  - Host-side numpy prep: degree/sym-normalization -> block-diag A, embedding
    gather h0 = (op_table + device_emb)[op_idx], fold ln_g/ln_b into W1/b1.
  - bf16 matmul inputs (fp32 matmul is 4x slower on PE), fp32 PSUM accum.

Per-block layer pipeline (all [128, 128] tiles, feature dim D=128):
  h (node-major, bf16)
    --PE: h.T @ A-->           aggT (feature-major, PSUM) -> SBUF bf16
    --PE: aggT.T @ Wg-->       pre-act (node-major, PSUM)
    --ACT relu-->              x (node-major, bf16)
    --DVE bn_stats/aggr-->     mean/var per node -> rsd, -mu*rsd
    --ACT x*rsd - mu*rsd-->    t (node-major, bf16)   [ln_g/ln_b folded into FFN]
    --PE transpose-->          tT (feature-major)
    --PE: W1c.T @ tT--> relu --> r1c (4 chunks of 128, feature-major)
    --PE: r1c.T @ W2c (accum)--> ffn (node-major, PSUM)
    --DVE t + ffn-->           h_next (node-major, bf16)
  Readout: PE h4.T @ blockdiag-ones -> per-graph feature sums -> PE fc_w dot
  -> ACT sigmoid(sum/9 + fc_b).
"""

import numpy as np
import ml_dtypes
from contextlib import ExitStack

import concourse.bass as bass
import concourse.mybir as mybir
import concourse.tile as tile
from concourse import bacc
from concourse.bass_utils import run_bass_kernel_spmd

BF16 = ml_dtypes.bfloat16
F32 = mybir.dt.float32
BF = mybir.dt.bfloat16

B = 16384        # graphs
NPG = 9          # nodes per graph
D = 128          # d_model
L = 4            # layers
NC = 8           # cores
GPB = 14         # graphs per 128-node block (126 nodes + 2 pad)
NB = 4           # blocks per macro tile
GPC = B // NC                      # 2048 graphs per core
BLOCKS = -(-GPC // GPB)            # 147
MACROS = -(-BLOCKS // NB)          # 37
BLOCKS_P = MACROS * NB             # 148 blocks per core (padded)
GSLOT = BLOCKS_P * GPB             # 2072 graph slots per core
LN_EPS = 1e-5


def _build_program(n_macros, use_bg, use_b1, use_cres, use_lng, fcb):
    """Trace + compile the per-core SPMD program. Returns the Bacc object."""
    nc = bacc.Bacc("TRN2", target_bir_lowering=False, debug=False, num_devices=NC)

    dram = {}

    def din(name, shape, dt):
        dram[name] = nc.dram_tensor(name, list(shape), dt, kind="ExternalInput").ap()
        return dram[name]

    h0_d = din("h0", (n_macros, 128, NB, D), BF)
    ab_d = din("Ab", (n_macros, 128, NB, 128), BF)
    wg_d = din("Wg", (128, L, 128), BF)
    w1_d = din("W1f", (128, L, 4, 128), BF)
    w2_d = din("W2c", (128, L, 4, 128), BF)
    mm_d = din("Mm", (128, GPB), BF)
    id_d = din("Id", (128, 128), BF)
    fcw_d = din("fcw", (128, 1), BF)
    if use_bg:
        bg_d = din("bg", (1, L, 128), BF)
    if use_b1:
        b1_d = din("b1f", (128, L, 4), F32)
    if use_cres:
        cr_d = din("cres", (1, L, 128), BF)
    if use_lng:
        lg_d = din("lng", (128, L, 128), F32)
    out_d = nc.dram_tensor("out", [1, n_macros * NB * GPB], F32,
                           kind="ExternalOutput").ap()

    AFT = mybir.ActivationFunctionType

    with tile.TileContext(nc) as tc, ExitStack() as ctx:
        const = ctx.enter_context(tc.tile_pool(name="const", bufs=1))
        io = ctx.enter_context(tc.tile_pool(name="io", bufs=3))
        work = ctx.enter_context(tc.tile_pool(name="work", bufs=2))
        stat = ctx.enter_context(tc.tile_pool(name="stat", bufs=2))
        psA = ctx.enter_context(tc.tile_pool(name="psA", bufs=1, space="PSUM"))
        psH = ctx.enter_context(tc.tile_pool(name="psH", bufs=1, space="PSUM"))
        psT = ctx.enter_context(tc.tile_pool(name="psT", bufs=1, space="PSUM"))
        ps1 = ctx.enter_context(tc.tile_pool(name="ps1", bufs=2, space="PSUM"))
        ps2 = ctx.enter_context(tc.tile_pool(name="ps2", bufs=1, space="PSUM"))
        psR = ctx.enter_context(tc.tile_pool(name="psR", bufs=1, space="PSUM"))

        # --- constants resident in SBUF ---
        wg_t = const.tile([128, L, 128], BF)
        nc.sync.dma_start(out=wg_t[:], in_=wg_d[:])
        w1_t = const.tile([128, L, 4, 128], BF)
        nc.sync.dma_start(out=w1_t[:], in_=w1_d[:])
        w2_t = const.tile([128, L, 4, 128], BF)
        nc.sync.dma_start(out=w2_t[:], in_=w2_d[:])
        mm_t = const.tile([128, GPB], BF)
        nc.sync.dma_start(out=mm_t[:], in_=mm_d[:])
        id_t = const.tile([128, 128], BF)
        nc.sync.dma_start(out=id_t[:], in_=id_d[:])
        fcw_t = const.tile([128, 1], BF)
        nc.sync.dma_start(out=fcw_t[:], in_=fcw_d[:])
        if use_bg or use_cres:
            ones_t = const.tile([1, 128], BF)
            nc.vector.memset(ones_t[:], 1.0)
        if use_bg:
            bg_t = const.tile([1, L, 128], BF)
            nc.sync.dma_start(out=bg_t[:], in_=bg_d[:])
        if use_b1:
            b1_t = const.tile([128, L, 4], F32)
            nc.sync.dma_start(out=b1_t[:], in_=b1_d[:])
        if use_cres:
            cr_t = const.tile([1, L, 128], BF)
            nc.sync.dma_start(out=cr_t[:], in_=cr_d[:])
        if use_lng:
            lg_t = const.tile([128, L, 128], F32)
            nc.sync.dma_start(out=lg_t[:], in_=lg_d[:])
        outsb = const.tile([1, n_macros * NB * GPB], F32)
        eps_t = const.tile([128, 1], F32)
        nc.vector.memset(eps_t[:], LN_EPS)
        fcb_t = const.tile([1, 1], F32)
        nc.vector.memset(fcb_t[:], fcb)

        for m in range(n_macros):
            h0_t = io.tile([128, NB, D], BF, tag="h0")
            nc.sync.dma_start(out=h0_t[:], in_=h0_d[m])
            a_t = io.tile([128, NB, 128], BF, tag="Ab")
            nc.sync.dma_start(out=a_t[:], in_=ab_d[m])

            hcur = h0_t
            for l in range(L):
                # ---- message passing: aggT[feat, dst] = h.T @ A ----
                pA = psA.tile([128, NB, 128], F32, tag="psA")
                for b in range(NB):
                    nc.tensor.matmul(pA[:, b, :], lhsT=hcur[:, b, :],
                                     rhs=a_t[:, b, :], start=True, stop=True)
                aggT = work.tile([128, NB, 128], BF, tag="aggT")
                nc.vector.tensor_copy(aggT[:], pA[:])

                # ---- GCN linear: pre[node, dout] = aggT.T @ Wg[l] ----
                pH = psH.tile([128, NB, 128], F32, tag="psH")
                for b in range(NB):
                    nc.tensor.matmul(pH[:, b, :], lhsT=aggT[:, b, :],
                                     rhs=wg_t[:, l, :], start=True,
                                     stop=not use_bg)
                    if use_bg:
                        nc.tensor.matmul(pH[:, b, :], lhsT=ones_t[:, :128],
                                         rhs=bg_t[:, l, :], start=False,
                                         stop=True)
                x_t = work.tile([128, NB, D], BF, tag="x")
                nc.scalar.activation(x_t[:], pH[:], AFT.Relu)

                # ---- layernorm stats (per node = per partition) ----
                bst = stat.tile([128, NB, 6], F32, tag="bst")
                mv = stat.tile([128, NB, 2], F32, tag="mv")
                for b in range(NB):
                    nc.vector.bn_stats(bst[:, b, :], x_t[:, b, :])
                    nc.vector.bn_aggr(mv[:, b, :], bst[:, b, :])
                sd = stat.tile([128, NB], F32, tag="sd")
                nc.scalar.activation(sd[:], mv[:, :, 1], AFT.Sqrt,
                                     bias=eps_t[:])
                rsd = stat.tile([128, NB], F32, tag="rsd")
                nc.vector.reciprocal(rsd[:], sd[:])
                nm = stat.tile([128, NB], F32, tag="nm")
                nc.vector.tensor_scalar_mul(nm[:], mv[:, :, 0], -1.0)
                nmrs = stat.tile([128, NB], F32, tag="nmrs")
                nc.vector.tensor_mul(nmrs[:], nm[:], rsd[:])

                # ---- t = (x - mu) * rsd  (ln_g/ln_b folded into FFN) ----
                t_t = work.tile([128, NB, D], BF, tag="t")
                for b in range(NB):
                    nc.scalar.activation(t_t[:, b, :], x_t[:, b, :],
                                         AFT.Identity,
                                         scale=rsd[:, b:b + 1],
                                         bias=nmrs[:, b:b + 1])

                # ---- transpose t -> tT (feature-major) ----
                pT = psT.tile([128, NB, 128], BF, tag="psT")
                for b in range(NB):
                    nc.tensor.transpose(pT[:, b, :], t_t[:, b, :], id_t[:])
                tT = work.tile([128, NB, 128], BF, tag="tT")
                nc.vector.tensor_copy(tT[:], pT[:])

                # ---- FFN ----
                p2 = ps2.tile([128, NB, 128], F32, tag="ps2")
                r1s = []
                for c in range(4):
                    p1 = ps1.tile([128, NB * 128], F32, tag="ps1")
                    nc.tensor.matmul(p1[:], lhsT=w1_t[:, l, c, :],
                                     rhs=tT[:],
                                     start=True, stop=True)
                    r1 = work.tile([128, NB * 128], BF, tag=f"r1_{c}")
                    if use_b1:
                        nc.vector.tensor_scalar(r1[:], p1[:],
                                                b1_t[:, l, c:c + 1], 0.0,
                                                op0=mybir.AluOpType.add,
                                                op1=mybir.AluOpType.max)
                    elif c % 2 == 0:
                        nc.vector.tensor_scalar_max(r1[:], p1[:], 0.0)
                    else:
                        nc.scalar.activation(r1[:], p1[:], AFT.Relu)
                    r1s.append(r1)
                for b in range(NB):
                    for c in range(4):
                        nc.tensor.matmul(p2[:, b, :],
                                         lhsT=r1s[c][:, b * 128:(b + 1) * 128],
                                         rhs=w2_t[:, l, c, :],
                                         start=(c == 0),
                                         stop=(c == 3 and not use_cres))
                    if use_cres:
                        nc.tensor.matmul(p2[:, b, :], lhsT=ones_t[:, :128],
                                         rhs=cr_t[:, l, :], start=False,
                                         stop=True)

                # ---- residual: h_next = t [*ln_g] + ffn ----
                hnext = work.tile([128, NB, D], BF, tag="h")
                if use_lng:
                    tg = work.tile([128, NB, D], F32, tag="tg")
                    for b in range(NB):
                        nc.vector.tensor_mul(tg[:, b, :], t_t[:, b, :],
                                             lg_t[:, l, :])
                    nc.vector.tensor_add(hnext[:], tg[:], p2[:])
                else:
                    nc.vector.tensor_add(hnext[:], t_t[:], p2[:])
                hcur = hnext

            # ---- readout: per-graph feature sums -> fc -> sigmoid ----
            pR = psR.tile([128, NB, GPB], F32, tag="psR")
            for b in range(NB):
                nc.tensor.matmul(pR[:, b, :], lhsT=hcur[:, b, :], rhs=mm_t[:],
                                 start=True, stop=True)
            gT = work.tile([128, NB, GPB], BF, tag="gT")
            nc.vector.tensor_copy(gT[:], pR[:])
            pF = psR.tile([1, NB * GPB], F32, tag="psF")
            nc.tensor.matmul(pF[:], lhsT=fcw_t[:],
                             rhs=gT[:],
                             start=True, stop=True)
            nc.scalar.activation(
                outsb[:, m * NB * GPB:(m + 1) * NB * GPB], pF[:],
                AFT.Sigmoid, scale=1.0 / NPG, bias=fcb_t[:])

        nc.sync.dma_start(out=out_d[:], in_=outsb[:])

    nc.compile()
    return nc


_CACHE = {}


def _prep_inputs(op_table, device_emb, Wg, bg, ln_g, ln_b, W1, b1, W2, b2,
                 fc_w, fc_b, op_idx, src, dst):
    """Host-side prep: returns (in_maps, flags)."""
    f32 = np.float32
    op_table = np.asarray(op_table, f32)
    device_emb = np.asarray(device_emb, f32)
    Wg = np.asarray(Wg, f32); bg = np.asarray(bg, f32)
    ln_g = np.asarray(ln_g, f32); ln_b = np.asarray(ln_b, f32)
    W1 = np.asarray(W1, f32); b1 = np.asarray(b1, f32)
    W2 = np.asarray(W2, f32); b2 = np.asarray(b2, f32)
    fc_w = np.asarray(fc_w, f32); fc_b = np.asarray(fc_b, f32)
    op_idx = np.asarray(op_idx); src = np.asarray(src); dst = np.asarray(dst)

    N = B * NPG
    # normalized adjacency with self loops, per graph
    deg = np.bincount(dst, minlength=N).astype(f32) + 1.0
    isd = (1.0 / np.sqrt(deg)).astype(f32)
    Agr = np.zeros((B, NPG, NPG), f32)
    np.add.at(Agr, (src // NPG, src % NPG, dst % NPG), isd[src] * isd[dst])
    bidx = np.arange(B)[:, None]
    nidx = np.arange(NPG)[None, :]
    Agr[bidx, nidx, nidx] += (isd * isd).reshape(B, NPG)

    # initial embeddings
    opT = op_table + device_emb            # [7, D]
    h0 = opT[op_idx.reshape(-1)]           # [N, D]

    # fold layernorm affine into FFN
    W1f = ln_g[:, :, None] * W1                              # [L, D, 4D]
    b1f = np.einsum('ld,ldk->lk', ln_b, W1) + b1             # [L, 4D]
    cres = ln_b + b2                                         # [L, D]

    use_bg = bool(np.any(bg))
    use_b1 = bool(np.any(b1f))
    use_cres = bool(np.any(cres))
    use_lng = bool(np.any(ln_g != 1.0))

    # per-core packing (pad graphs to GSLOT, nodes to 128 per block)
    h0g = np.zeros((NC, GSLOT, NPG, D), f32)
    h0g[:, :GPC] = h0.reshape(NC, GPC, NPG, D)
    h0p = np.zeros((NC, BLOCKS_P, 128, D), f32)
    h0p[:, :, :GPB * NPG, :] = h0g.reshape(NC, BLOCKS_P, GPB * NPG, D)
    h0c = np.ascontiguousarray(
        h0p.reshape(NC, MACROS, NB, 128, D).transpose(0, 1, 3, 2, 4)
    ).astype(BF16)

    Ag = np.zeros((NC, GSLOT, NPG, NPG), f32)
    Ag[:, :GPC] = Agr.reshape(NC, GPC, NPG, NPG)
    Ab = np.zeros((NC, BLOCKS_P, 128, 128), f32)
    Agb = Ag.reshape(NC, BLOCKS_P, GPB, NPG, NPG)
    for i in range(GPB):
        Ab[:, :, NPG * i:NPG * (i + 1), NPG * i:NPG * (i + 1)] = Agb[:, :, i]
    Abc = np.ascontiguousarray(
        Ab.reshape(NC, MACROS, NB, 128, 128).transpose(0, 1, 3, 2, 4)
    ).astype(BF16)

    # weights (shared across cores)
    wg_h = np.ascontiguousarray(Wg.transpose(1, 0, 2)).astype(BF16)
    w1_h = np.ascontiguousarray(
        W1f.reshape(L, D, 4, 128).transpose(1, 0, 2, 3)).astype(BF16)
    w2_h = np.ascontiguousarray(
        W2.reshape(L, 4, 128, D).transpose(2, 0, 1, 3)).astype(BF16)
    mm_h = np.zeros((128, GPB), BF16)
    for g in range(GPB):
        mm_h[NPG * g:NPG * (g + 1), g] = 1.0
    id_h = np.eye(128, dtype=BF16)
    fcw_h = fc_w.astype(BF16)

    shared = dict(Wg=wg_h, W1f=w1_h, W2c=w2_h, Mm=mm_h, Id=id_h, fcw=fcw_h)
    if use_bg:
        shared["bg"] = np.ascontiguousarray(bg[None]).astype(BF16)
    if use_b1:
        shared["b1f"] = np.ascontiguousarray(
            b1f.reshape(L, 4, 128).transpose(2, 0, 1)).astype(f32)
    if use_cres:
        shared["cres"] = np.ascontiguousarray(cres[None]).astype(BF16)
    if use_lng:
        shared["lng"] = np.ascontiguousarray(ln_g.transpose(1, 0)[:, :, None]
                                             * np.ones((1, 1, 128), f32))
        # [128?] -> broadcast ln_g over feat? ln_g is per-feature: tile is
        # [128(node slot), L, 128(feat)] with value ln_g[l, f] for all slots
        shared["lng"] = np.ascontiguousarray(
            np.broadcast_to(ln_g[None, :, :], (128, L, 128))).astype(f32)

    in_maps = [dict(shared, h0=h0c[c], Ab=Abc[c]) for c in range(NC)]
    flags = (use_bg, use_b1, use_cres, use_lng)
    return in_maps, flags, float(fc_b.reshape(-1)[0])


_LAST_IN_MAPS = None


def kernel(**inputs):
    global _LAST_IN_MAPS
    in_maps, flags, fcb = _prep_inputs(**inputs)
    _LAST_IN_MAPS = in_maps
    key = (MACROS,) + flags + (fcb,)
    if key not in _CACHE:
        _CACHE[key] = _build_program(MACROS, *flags, fcb)
    nc = _CACHE[key]
    res = run_bass_kernel_spmd(nc, in_maps, list(range(NC)))
    out = np.empty((B, 1), np.float32)
    for c in range(NC):
        out[c * GPC:(c + 1) * GPC, 0] = res.results[c]["out"][0, :GPC]
    return out


# revision 29
# speedup vs baseline: 11.3917x; 11.3917x over previous
"""Trainium2 Bass kernel for nn_Backbone_4243427688698 (gnn_message_passing).

Strategy (data-parallel over graphs, 8 cores):
  - Each of the 8 cores processes 2048 whole 9-node graphs.
  - Edges never cross graphs, so message passing for a group of 14 graphs
    (126 nodes, padded to 128) is a dense block-diagonal 128x128 adjacency
    matmul on the TensorEngine.
  - The full 4-layer GCN for a block is fused in SBUF: only the initial node
    embeddings h0 and the normalized adjacency blocks stream in from HBM; the
    only output is the [B, 1] sigmoid readout.
  - Host-side numpy prep: degree/sym-normalization -> block-diag A, embedding
    gather h0 = (op_table + device_emb)[op_idx], fold ln_g/ln_b into W1/b1.
  - bf16 matmul inputs (fp32 matmul is 4x slower on PE), fp32 PSUM accum.

Per-block layer pipeline (all [128, 128] tiles, feature dim D=128):
  h (node-major, bf16)
    --PE: h.T @ A-->           aggT (feature-major, PSUM) -> SBUF bf16
    --PE: aggT.T @ Wg-->       pre-act (node-major, PSUM)
    --ACT relu-->              x (node-major, bf16)
    --DVE bn_stats/aggr-->     mean/var per node -> rsd, -mu*rsd
    --ACT x*rsd - mu*rsd-->    t (node-major, bf16)   [ln_g/ln_b folded into FFN]
    --PE transpose-->          tT (feature-major)
    --PE: W1c.T @ tT--> relu --> r1c (4 chunks of 128, feature-major)
    --PE: r1c.T @ W2c (accum)--> ffn (node-major, PSUM)
    --DVE t + ffn-->           h_next (node-major, bf16)
  Readout: PE h4.T @ blockdiag-ones -> per-graph feature sums -> PE fc_w dot
  -> ACT sigmoid(sum/9 + fc_b).
"""

import numpy as np
import ml_dtypes
from contextlib import ExitStack

import concourse.bass as bass
import concourse.mybir as mybir
import concourse.tile as tile
from concourse import bacc
from concourse.bass_utils import run_bass_kernel_spmd

BF16 = ml_dtypes.bfloat16
F32 = mybir.dt.float32
BF = mybir.dt.bfloat16

B = 16384        # graphs
NPG = 9          # nodes per graph
D = 128          # d_model
L = 4            # layers
NC = 8           # cores
GPB = 14         # graphs per 128-node block (126 nodes + 2 pad)
NB = 4           # blocks per macro tile
GPC = B // NC                      # 2048 graphs per core
BLOCKS = -(-GPC // GPB)            # 147
MACROS = -(-BLOCKS // NB)          # 37
BLOCKS_P = MACROS * NB             # 148 blocks per core (padded)
GSLOT = BLOCKS_P * GPB             # 2072 graph slots per core
LN_EPS = 1e-5


def _build_program(n_macros, use_bg, use_b1, use_cres, use_lng, fcb):
    """Trace + compile the per-core SPMD program. Returns the Bacc object."""
    nc = bacc.Bacc("TRN2", target_bir_lowering=False, debug=False, num_devices=NC)

    dram = {}

    def din(name, shape, dt):
        dram[name] = nc.dram_tensor(name, list(shape), dt, kind="ExternalInput").ap()
        return dram[name]

    h0_d = din("h0", (n_macros, 128, NB, D), BF)
    ab_d = din("Ab", (n_macros, 128, NB, 128), BF)
    wg_d = din("Wg", (128, L, 128), BF)
    w1_d = din("W1f", (128, L, 4, 128), BF)
    w2_d = din("W2c", (128, L, 4, 128), BF)
    mm_d = din("Mm", (128, GPB), BF)
    id_d = din("Id", (128, 128), BF)
    fcw_d = din("fcw", (128, 1), BF)
    if use_bg:
        bg_d = din("bg", (1, L, 128), BF)
    if use_b1:
        b1_d = din("b1f", (128, L, 4), F32)
    if use_cres:
        cr_d = din("cres", (1, L, 128), BF)
    if use_lng:
        lg_d = din("lng", (128, L, 128), BF)
    out_d = nc.dram_tensor("out", [1, n_macros * NB * GPB], F32,
                           kind="ExternalOutput").ap()

    AFT = mybir.ActivationFunctionType

    with tile.TileContext(nc) as tc, ExitStack() as ctx:
        const = ctx.enter_context(tc.tile_pool(name="const", bufs=1))
        io = ctx.enter_context(tc.tile_pool(name="io", bufs=11))
        work = ctx.enter_context(tc.tile_pool(name="work", bufs=4))
        stat = ctx.enter_context(tc.tile_pool(name="stat", bufs=4))
        hpool = ctx.enter_context(tc.tile_pool(name="hpool", bufs=11))
        # PSUM: 8 banks total. pA/pH have strictly sequential lifetimes
        # (Wg-mult depends on aggT which depends on pA), so they share one
        # tag; same for pT (transpose) and pR/pF (readout).
        psAH = ctx.enter_context(tc.tile_pool(name="psAH", bufs=3, space="PSUM"))
        ps1 = ctx.enter_context(tc.tile_pool(name="ps1", bufs=2, space="PSUM"))
        ps2 = ctx.enter_context(tc.tile_pool(name="ps2", bufs=3, space="PSUM"))

        # --- constants resident in SBUF ---
        wg_t = const.tile([128, L, 128], BF)
        nc.sync.dma_start(out=wg_t[:], in_=wg_d[:])
        w1_t = const.tile([128, L, 4, 128], BF)
        nc.sync.dma_start(out=w1_t[:], in_=w1_d[:])
        w2_t = const.tile([128, L, 4, 128], BF)
        nc.sync.dma_start(out=w2_t[:], in_=w2_d[:])
        mm_t = const.tile([128, GPB], BF)
        nc.sync.dma_start(out=mm_t[:], in_=mm_d[:])
        id_t = const.tile([128, 128], BF)
        nc.sync.dma_start(out=id_t[:], in_=id_d[:])
        fcw_t = const.tile([128, 1], BF)
        nc.sync.dma_start(out=fcw_t[:], in_=fcw_d[:])
        if use_bg or use_cres:
            ones_t = const.tile([1, 128], BF)
            nc.vector.memset(ones_t[:], 1.0)
        if use_bg:
            bg_t = const.tile([1, L, 128], BF)
            nc.sync.dma_start(out=bg_t[:], in_=bg_d[:])
        if use_b1:
            b1_t = const.tile([128, L, 4], F32)
            nc.sync.dma_start(out=b1_t[:], in_=b1_d[:])
        if use_cres:
            cr_t = const.tile([1, L, 128], BF)
            nc.sync.dma_start(out=cr_t[:], in_=cr_d[:])
        if use_lng:
            lg_t = const.tile([128, L, 128], BF)
            nc.sync.dma_start(out=lg_t[:], in_=lg_d[:])
        outsb = const.tile([1, n_macros * NB * GPB], F32)
        eps_t = const.tile([128, 1], F32)
        nc.vector.memset(eps_t[:], LN_EPS)
        fcb_t = const.tile([1, 1], F32)
        nc.vector.memset(fcb_t[:], fcb)

        # Macros are processed in stage-interleaved pairs: each engine's
        # in-order instruction stream alternates between two independent
        # macros, so one macro's compute fills the other's dependency stalls.
        st = {}  # per-macro in-flight state

        def stage_load(m):
            h0_t = hpool.tile([128, NB, D], BF, tag="h")
            nc.sync.dma_start(out=h0_t[:], in_=h0_d[m])
            a_t = io.tile([128, NB, 128], BF, tag="Ab")
            nc.sync.dma_start(out=a_t[:], in_=ab_d[m])
            st[m] = dict(h=h0_t, a=a_t)

        def stage_agg(m, l):
            s = st[m]
            pA = psAH.tile([128, NB, 128], F32, tag="psAH")
            for b in range(NB):
                nc.tensor.matmul(pA[:, b, :], lhsT=s["h"][:, b, :],
                                 rhs=s["a"][:, b, :], start=True, stop=True)
            aggT = work.tile([128, NB, 128], BF, tag="aggT")
            if l % 2 == 0:
                nc.vector.tensor_copy(aggT[:], pA[:])
            else:
                nc.scalar.copy(aggT[:], pA[:])
            s["aggT"] = aggT

        def stage_wg(m, l):
            s = st[m]
            pH = psAH.tile([128, NB, 128], F32, tag="psAH")
            for b in range(NB):
                nc.tensor.matmul(pH[:, b, :], lhsT=s["aggT"][:, b, :],
                                 rhs=wg_t[:, l, :], start=True,
                                 stop=not use_bg)
                if use_bg:
                    nc.tensor.matmul(pH[:, b, :], lhsT=ones_t[:, :128],
                                     rhs=bg_t[:, l, :], start=False,
                                     stop=True)
            x_t = work.tile([128, NB, D], BF, tag="x")
            nc.scalar.activation(x_t[:], pH[:], AFT.Relu)
            s["x"] = x_t

        def stage_stats(m):
            s = st[m]
            x_t = s["x"]
            bst = stat.tile([128, NB, 6], F32, tag="bst")
            mv = stat.tile([128, NB, 2], F32, tag="mv")
            for b in range(NB):
                nc.vector.bn_stats(bst[:, b, :], x_t[:, b, :])
                nc.vector.bn_aggr(mv[:, b, :], bst[:, b, :])
            sd = stat.tile([128, NB], F32, tag="sd")
            nc.scalar.activation(sd[:], mv[:, :, 1], AFT.Sqrt, bias=eps_t[:])
            rsd = stat.tile([128, NB], F32, tag="rsd")
            nc.vector.reciprocal(rsd[:], sd[:])
            nm = stat.tile([128, NB], F32, tag="nm")
            nc.gpsimd.tensor_scalar_mul(nm[:], mv[:, :, 0], -1.0)
            s["rsd"], s["nm"] = rsd, nm

        def stage_t(m, l):
            s = st[m]
            t_t = work.tile([128, NB, D], BF, tag="t")
            t_eng = [nc.gpsimd, nc.gpsimd, nc.gpsimd, nc.gpsimd]
            for b in range(NB):
                t_eng[b].tensor_scalar(t_t[:, b, :], s["x"][:, b, :],
                                       s["nm"][:, b:b + 1],
                                       s["rsd"][:, b:b + 1],
                                       op0=mybir.AluOpType.add,
                                       op1=mybir.AluOpType.mult)
            pT = ps2.tile([128, NB, 128], BF, tag="ps2")
            for b in range(NB):
                nc.tensor.transpose(pT[:, b, :], t_t[:, b, :], id_t[:])
            tT = work.tile([128, NB, 128], BF, tag="tT")
            if l % 2 == 0:
                nc.scalar.copy(tT[:], pT[:])
            else:
                nc.vector.tensor_copy(tT[:], pT[:])
            s["tT"] = tT

        def stage_w1(m, l, c):
            s = st[m]
            p1 = ps1.tile([128, NB * 128], F32, tag="ps1")
            nc.tensor.matmul(p1[:], lhsT=w1_t[:, l, c, :], rhs=s["tT"][:],
                             start=True, stop=True)
            r1 = work.tile([128, NB * 128], BF, tag=f"r1_{c}")
            if use_b1:
                nc.vector.tensor_scalar(r1[:], p1[:], b1_t[:, l, c:c + 1],
                                        0.0, op0=mybir.AluOpType.add,
                                        op1=mybir.AluOpType.max)
            elif c == 0 or (c == 2 and l % 2 == 0):
                nc.vector.tensor_scalar_max(r1[:], p1[:], 0.0)
            else:
                nc.scalar.activation(r1[:], p1[:], AFT.Relu)
            s.setdefault("r1", {})[c] = r1

        def stage_w2(m, l):
            s = st[m]
            p2 = ps2.tile([128, NB, 128], F32, tag="ps2")
            for b in range(NB):
                for c in range(4):
                    nc.tensor.matmul(p2[:, b, :],
                                     lhsT=s["r1"][c][:, b * 128:(b + 1) * 128],
                                     rhs=w2_t[:, l, c, :], start=(c == 0),
                                     stop=False)
                # residual on PE: p2 += tT.T @ I  (or @ diag(ln_g) in general)
                res_rhs = lg_t[:, l, :] if use_lng else id_t[:]
                nc.tensor.matmul(p2[:, b, :], lhsT=s["tT"][:, b, :],
                                 rhs=res_rhs, start=False, stop=not use_cres)
                if use_cres:
                    nc.tensor.matmul(p2[:, b, :], lhsT=ones_t[:, :128],
                                     rhs=cr_t[:, l, :], start=False, stop=True)
            hnext = hpool.tile([128, NB, D], BF, tag="h")
            nc.vector.tensor_copy(hnext[:], p2[:])
            s["h"] = hnext

        def stage_readout(m):
            s = st[m]
            pR = ps2.tile([128, NB, GPB], F32, tag="ps2")
            for b in range(NB):
                nc.tensor.matmul(pR[:, b, :], lhsT=s["h"][:, b, :],
                                 rhs=mm_t[:], start=True, stop=True)
            gT = work.tile([128, NB, GPB], BF, tag="gT")
            nc.vector.tensor_copy(gT[:], pR[:])
            pF = ps2.tile([1, NB * GPB], F32, tag="ps2")
            nc.tensor.matmul(pF[:], lhsT=fcw_t[:], rhs=gT[:],
                             start=True, stop=True)
            # logits accumulate in SBUF; one sigmoid at the end avoids ACT
            # function-table swaps (sqrt and sigmoid live in different sets)
            nc.scalar.activation(
                outsb[:, m * NB * GPB:(m + 1) * NB * GPB], pF[:],
                AFT.Identity, scale=1.0 / NPG, bias=fcb_t[:])
            del st[m]

        # Four-deep software pipeline over (macro, layer) units: at each
        # tick, P3 (W2+residual) of unit k-3, P2 (W1+relu) of k-2, P1 (LN)
        # of k-1 and P0 (aggregate + GCN linear) of k are emitted. The
        # h-recurrence (m,l)->(m,l+1) is 4 units apart, matching the depth.
        # Rotation chunks must be at least DEPTH macros wide so that the
        # h-recurrence (m,l)->(m,l+1) is emitted after the pipeline writes
        # h; narrow chunks are padded with bubble slots (None).
        DEPTH = 4
        R = 9
        n_chunks = max(1, -(-n_macros // R))
        base = n_macros // n_chunks
        sizes = [base + (1 if i < n_macros % n_chunks else 0)
                 for i in range(n_chunks)]
        stream = []
        m0 = 0
        for sz in sizes:
            gm = list(range(m0, m0 + sz))
            m0 += sz
            pad = max(0, DEPTH - sz)
            for l in range(L):
                for m in gm:
                    stream.append((m, l))
                stream.extend([None] * pad)
        first_tick = {}
        for k, u in enumerate(stream):
            if u is not None and u[1] == 0:
                first_tick[u[0]] = k

        nk = len(stream)
        for k in range(nk + 3):
            for m, t0 in first_tick.items():
                if t0 == k + 2 or (t0 < 2 and k == 0 and m not in st):
                    stage_load(m)
            if k >= 3 and stream[k - 3] is not None:
                m, l = stream[k - 3]
                stage_w2(m, l)
                if l == L - 1:
                    stage_readout(m)
            if 2 <= k < nk + 2 and stream[k - 2] is not None:
                m, l = stream[k - 2]
                for c in range(4):
                    stage_w1(m, l, c)
            if 1 <= k < nk + 1 and stream[k - 1] is not None:
                m, l = stream[k - 1]
                stage_stats(m)
                stage_t(m, l)
            if k < nk and stream[k] is not None:
                m, l = stream[k]
                stage_agg(m, l)
                stage_wg(m, l)

        nc.scalar.activation(outsb[:], outsb[:], AFT.Sigmoid)
        nc.sync.dma_start(out=out_d[:], in_=outsb[:])

    nc.compile()
    return nc


_CACHE = {}


def _prep_inputs(op_table, device_emb, Wg, bg, ln_g, ln_b, W1, b1, W2, b2,
                 fc_w, fc_b, op_idx, src, dst):
    """Host-side prep: returns (in_maps, flags)."""
    f32 = np.float32
    op_table = np.asarray(op_table, f32)
    device_emb = np.asarray(device_emb, f32)
    Wg = np.asarray(Wg, f32); bg = np.asarray(bg, f32)
    ln_g = np.asarray(ln_g, f32); ln_b = np.asarray(ln_b, f32)
    W1 = np.asarray(W1, f32); b1 = np.asarray(b1, f32)
    W2 = np.asarray(W2, f32); b2 = np.asarray(b2, f32)
    fc_w = np.asarray(fc_w, f32); fc_b = np.asarray(fc_b, f32)
    op_idx = np.asarray(op_idx); src = np.asarray(src); dst = np.asarray(dst)

    N = B * NPG
    # normalized adjacency with self loops, per graph
    deg = np.bincount(dst, minlength=N).astype(f32) + 1.0
    isd = (1.0 / np.sqrt(deg)).astype(f32)
    Agr = np.zeros((B, NPG, NPG), f32)
    np.add.at(Agr, (src // NPG, src % NPG, dst % NPG), isd[src] * isd[dst])
    bidx = np.arange(B)[:, None]
    nidx = np.arange(NPG)[None, :]
    Agr[bidx, nidx, nidx] += (isd * isd).reshape(B, NPG)

    # initial embeddings
    opT = op_table + device_emb            # [7, D]
    h0 = opT[op_idx.reshape(-1)]           # [N, D]

    # fold layernorm affine into FFN
    W1f = ln_g[:, :, None] * W1                              # [L, D, 4D]
    b1f = np.einsum('ld,ldk->lk', ln_b, W1) + b1             # [L, 4D]
    cres = ln_b + b2                                         # [L, D]

    use_bg = bool(np.any(bg))
    use_b1 = bool(np.any(b1f))
    use_cres = bool(np.any(cres))
    use_lng = bool(np.any(ln_g != 1.0))

    # per-core packing (pad graphs to GSLOT, nodes to 128 per block)
    h0g = np.zeros((NC, GSLOT, NPG, D), f32)
    h0g[:, :GPC] = h0.reshape(NC, GPC, NPG, D)
    h0p = np.zeros((NC, BLOCKS_P, 128, D), f32)
    h0p[:, :, :GPB * NPG, :] = h0g.reshape(NC, BLOCKS_P, GPB * NPG, D)
    h0c = np.ascontiguousarray(
        h0p.reshape(NC, MACROS, NB, 128, D).transpose(0, 1, 3, 2, 4)
    ).astype(BF16)

    Ag = np.zeros((NC, GSLOT, NPG, NPG), f32)
    Ag[:, :GPC] = Agr.reshape(NC, GPC, NPG, NPG)
    Ab = np.zeros((NC, BLOCKS_P, 128, 128), f32)
    Agb = Ag.reshape(NC, BLOCKS_P, GPB, NPG, NPG)
    for i in range(GPB):
        Ab[:, :, NPG * i:NPG * (i + 1), NPG * i:NPG * (i + 1)] = Agb[:, :, i]
    Abc = np.ascontiguousarray(
        Ab.reshape(NC, MACROS, NB, 128, 128).transpose(0, 1, 3, 2, 4)
    ).astype(BF16)

    # weights (shared across cores)
    wg_h = np.ascontiguousarray(Wg.transpose(1, 0, 2)).astype(BF16)
    w1_h = np.ascontiguousarray(
        W1f.reshape(L, D, 4, 128).transpose(1, 0, 2, 3)).astype(BF16)
    w2_h = np.ascontiguousarray(
        W2.reshape(L, 4, 128, D).transpose(2, 0, 1, 3)).astype(BF16)
    mm_h = np.zeros((128, GPB), BF16)
    for g in range(GPB):
        mm_h[NPG * g:NPG * (g + 1), g] = 1.0
    id_h = np.eye(128, dtype=BF16)
    fcw_h = fc_w.astype(BF16)

    shared = dict(Wg=wg_h, W1f=w1_h, W2c=w2_h, Mm=mm_h, Id=id_h, fcw=fcw_h)
    if use_bg:
        shared["bg"] = np.ascontiguousarray(bg[None]).astype(BF16)
    if use_b1:
        shared["b1f"] = np.ascontiguousarray(
            b1f.reshape(L, 4, 128).transpose(2, 0, 1)).astype(f32)
    if use_cres:
        shared["cres"] = np.ascontiguousarray(cres[None]).astype(BF16)
    if use_lng:
        # diag(ln_g[l]) matrices: residual-on-PE computes p2 += tT.T @ diag(g)
        dg = np.zeros((128, L, 128), f32)
        for l in range(L):
            dg[np.arange(D), l, np.arange(D)] = ln_g[l]
        shared["lng"] = dg.astype(BF16)

    in_maps = [dict(shared, h0=h0c[c], Ab=Abc[c]) for c in range(NC)]
    flags = (use_bg, use_b1, use_cres, use_lng)
    return in_maps, flags, float(fc_b.reshape(-1)[0])


_LAST_IN_MAPS = None


def kernel(**inputs):
    global _LAST_IN_MAPS
    in_maps, flags, fcb = _prep_inputs(**inputs)
    _LAST_IN_MAPS = in_maps
    key = (MACROS,) + flags + (fcb,)
    if key not in _CACHE:
        _CACHE[key] = _build_program(MACROS, *flags, fcb)
    nc = _CACHE[key]
    res = run_bass_kernel_spmd(nc, in_maps, list(range(NC)))
    out = np.empty((B, 1), np.float32)
    for c in range(NC):
        out[c * GPC:(c + 1) * GPC, 0] = res.results[c]["out"][0, :GPC]
    return out
